# revision 8
# baseline (speedup 1.0000x reference)
"""Trainium2 Bass kernel for a top-2-of-4 routed LSTM cell bank (MoE routing).

Reference computation (per batch row b):
    feats    = concat(x[b], h[b])                      # [512]
    logits   = feats @ W_ctrl + b_ctrl                 # [4]
    gate     = top2_softmax(logits)                    # [4], 2 nonzero
    combined = feats @ W_gates + b_gates               # [4 cells, 4 gates, 256]
    i, j, f, o = gates;  new_c_n = sig(f)*c + sig(i)*tanh(j);  new_h_n = sig(o)*tanh(new_c_n)
    nh[b] = sum_n gate[n]*new_h_n ; nc[b] = sum_n gate[n]*new_c_n

Strategy: data-parallel over 8 NeuronCores (2048 batch rows each), weights
replicated.  Per core everything is dense and batch-tiled (16 tiles of 128
rows).  The routing logits are computed with true-fp32 matmuls (the smallest
top2/top3 logit gap in the dataset is ~2e-5, so reduced precision would flip
routing decisions); the big [2048,512]@[512,4096] gate matmul runs in bf16
(full PE stream rate; fp32 matmul is 4 cycles/row).  W_gates columns are
permuted host-side to gate-major [i|f|o|j] order so each activation function
covers one contiguous span per batch tile.  The heavy elementwise chain is
split across DVE and GPSIMD; new_c/new_h live in one tile so the routed
combine handles both outputs per instruction.
"""

import sys

for _p in ("/opt/trn_rl_repo", "/root/.axon_site/_ro/trn_rl_repo"):
    if _p not in sys.path:
        sys.path.append(_p)

import numpy as np

import concourse.bacc as bacc
from concourse import bass, mybir
from concourse.bass_utils import run_bass_kernel_spmd
from concourse.tile import TileContext

P = 128
N_CORES = 8
B = 16384
IN = 256
OUT = 256
NCELL = 4
D = IN + OUT          # 512
KT = D // P           # 4 contraction tiles
BL = B // N_CORES     # 2048 rows per core
NT = BL // P          # 16 batch tiles per core
GC = 4 * OUT * NCELL  # 4096 gate columns

F32 = mybir.dt.float32
BF16 = mybir.dt.bfloat16
U32 = mybir.dt.uint32
I32 = mybir.dt.int32
AF = mybir.ActivationFunctionType
OP = mybir.AluOpType

# test.py can flip these to capture a profiled run
TRACE = False
LAST_RESULTS = None


def _build_program(has_bg: bool, has_bc: bool):
    nc = bacc.Bacc("TRN2", target_bir_lowering=False, debug=False,
                   num_devices=N_CORES)

    featsT = nc.dram_tensor("featsT", (D, BL), F32, kind="ExternalInput").ap()
    featsTb = nc.dram_tensor("featsTb", (D, BL), BF16, kind="ExternalInput").ap()
    wgb = nc.dram_tensor("wgb", (D, GC), BF16, kind="ExternalInput").ap()
    c_in = nc.dram_tensor("c_in", (BL, OUT), F32, kind="ExternalInput").ap()
    wc = nc.dram_tensor("wc", (D, NCELL), F32, kind="ExternalInput").ap()
    bg = bc = None
    if has_bg:
        bg = nc.dram_tensor("bg", (1, GC), F32, kind="ExternalInput").ap()
    if has_bc:
        bc = nc.dram_tensor("bc", (1, NCELL), F32, kind="ExternalInput").ap()
    nh_o = nc.dram_tensor("nh_out", (BL, OUT), F32, kind="ExternalOutput").ap()
    nc_o = nc.dram_tensor("nc_out", (BL, OUT), F32, kind="ExternalOutput").ap()

    with TileContext(nc) as tc:
        with tc.tile_pool(name="const", bufs=1) as konst, \
             tc.tile_pool(name="work", bufs=2) as work:

            # ---- input loads; one HWDGE FIFO orders availability:
            # fp32 featsT (logits) -> bf16 featsT -> W quarters -> c
            fT_sb = []
            for k in range(KT):
                t = konst.tile([P, BL], F32, tag=f"fT{k}")
                nc.sync.dma_start(
                    out=t[:], in_=featsT.rearrange("(k p) b -> k p b", p=P)[k])
                fT_sb.append(t)
            wc_sb = konst.tile([P, KT * NCELL], F32, tag="wc")
            nc.sync.dma_start(
                out=wc_sb[:].rearrange("p (k n) -> p k n", n=NCELL),
                in_=wc.rearrange("(k p) n -> p k n", p=P))
            fTb_sb = []
            for k in range(KT):
                t = konst.tile([P, BL], BF16, tag=f"fTb{k}", name=f"fTb_sb{k}")
                nc.sync.dma_start(
                    out=t[:], in_=featsTb.rearrange("(k p) b -> k p b", p=P)[k])
                fTb_sb.append(t)
            wg_sb = [konst.tile([P, GC], BF16, tag=f"wg{k}", name=f"wg_sb{k}")
                     for k in range(KT)]
            wg_dram = wgb.rearrange("(k p) n -> k p n", p=P)
            for q in range(4):                      # quarter-major: early cols first
                for k in range(KT):
                    nc.sync.dma_start(
                        out=wg_sb[k][:, q * 1024:(q + 1) * 1024],
                        in_=wg_dram[k][:, q * 1024:(q + 1) * 1024])
            c_sb = konst.tile([P, NT * OUT], F32, tag="c")
            nc.sync.dma_start(
                out=c_sb[:].rearrange("p (t o) -> p t o", o=OUT),
                in_=c_in.rearrange("(t p) o -> p t o", p=P))
            bg_sb = bc_sb = None
            if has_bg:
                bg_sb = konst.tile([P, GC], F32, tag="bg")
                nc.sync.dma_start(out=bg_sb[:], in_=bg.partition_broadcast(P)[:, 0, :])
            if has_bc:
                bc_sb = konst.tile([P, NCELL], F32, tag="bc")
                nc.sync.dma_start(out=bc_sb[:], in_=bc.partition_broadcast(P)[:, 0, :])

            # ---- phase A: routing logits (true fp32) + top-2 gates ----
            lg = konst.tile([P, NT * NCELL], F32, tag="lg")
            with tc.tile_pool(name="psA", bufs=1, space="PSUM") as psA:
                lg_ps = psA.tile([P, NT * NCELL], F32, tag="lps")
                for t_ in range(NT):
                    for k in range(KT):
                        nc.tensor.matmul(
                            lg_ps[:, t_ * NCELL:(t_ + 1) * NCELL],
                            lhsT=fT_sb[k][:, t_ * P:(t_ + 1) * P],
                            rhs=wc_sb[:, k * NCELL:(k + 1) * NCELL],
                            start=(k == 0), stop=(k == KT - 1))
                nc.vector.tensor_copy(lg[:], lg_ps[:])
            if has_bc:
                nc.vector.tensor_tensor(
                    out=lg[:].rearrange("p (t n) -> p t n", n=NCELL),
                    in0=lg[:].rearrange("p (t n) -> p t n", n=NCELL),
                    in1=bc_sb[:].unsqueeze(1).to_broadcast((P, NT, NCELL)),
                    op=OP.add)

            l8 = konst.tile([P, NT * 8], F32, tag="l8")
            nc.vector.memset(l8[:], -1e30)
            nc.vector.tensor_copy(
                out=l8[:].rearrange("p (t e) -> p t e", e=8)[:, :, 0:NCELL],
                in_=lg[:].rearrange("p (t n) -> p t n", n=NCELL))
            mx8 = konst.tile([P, NT * 8], F32, tag="mx8")
            ix8 = konst.tile([P, NT * 8], U32, tag="ix8")
            for t_ in range(NT):
                nc.vector.max(mx8[:, t_ * 8:(t_ + 1) * 8], l8[:, t_ * 8:(t_ + 1) * 8])
                nc.vector.max_index(ix8[:, t_ * 8:(t_ + 1) * 8],
                                    mx8[:, t_ * 8:(t_ + 1) * 8],
                                    l8[:, t_ * 8:(t_ + 1) * 8])
            mx_v = mx8[:].rearrange("p (t e) -> p t e", e=8)
            ix_v = ix8[:].rearrange("p (t e) -> p t e", e=8)

            diff = konst.tile([P, NT], F32, tag="diff")
            nc.vector.tensor_tensor(out=diff[:].unsqueeze(2),
                                    in0=mx_v[:, :, 0:1], in1=mx_v[:, :, 1:2],
                                    op=OP.subtract)
            p1 = konst.tile([P, NT], F32, tag="p1")
            nc.scalar.activation(p1[:], diff[:], AF.Sigmoid)
            p2 = konst.tile([P, NT], F32, tag="p2")
            nc.vector.tensor_scalar(p2[:], p1[:], -1.0, 1.0, OP.mult, OP.add)

            i1f = konst.tile([P, NT], F32, tag="i1f")
            i2f = konst.tile([P, NT], F32, tag="i2f")
            nc.vector.tensor_copy(i1f[:].unsqueeze(2), ix_v[:, :, 0:1])
            nc.vector.tensor_copy(i2f[:].unsqueeze(2), ix_v[:, :, 1:2])

            iota_i = konst.tile([P, NT * NCELL], I32, tag="iota_i")
            nc.gpsimd.iota(iota_i[:], pattern=[[0, NT], [1, NCELL]],
                           base=0, channel_multiplier=0)
            iota_f = konst.tile([P, NT * NCELL], F32, tag="iota_f")
            nc.vector.tensor_copy(iota_f[:], iota_i[:])
            iota_v = iota_f[:].rearrange("p (t n) -> p t n", n=NCELL)

            gate = konst.tile([P, NT * NCELL], F32, tag="gate")
            g2 = konst.tile([P, NT * NCELL], F32, tag="g2")
            gate_v = gate[:].rearrange("p (t n) -> p t n", n=NCELL)
            g2_v = g2[:].rearrange("p (t n) -> p t n", n=NCELL)
            nc.vector.tensor_tensor(
                out=gate_v, in0=i1f[:].unsqueeze(2).to_broadcast((P, NT, NCELL)),
                in1=iota_v, op=OP.is_equal)
            nc.vector.tensor_tensor(
                out=gate_v, in0=gate_v,
                in1=p1[:].unsqueeze(2).to_broadcast((P, NT, NCELL)), op=OP.mult)
            nc.vector.tensor_tensor(
                out=g2_v, in0=i2f[:].unsqueeze(2).to_broadcast((P, NT, NCELL)),
                in1=iota_v, op=OP.is_equal)
            nc.vector.tensor_tensor(
                out=g2_v, in0=g2_v,
                in1=p2[:].unsqueeze(2).to_broadcast((P, NT, NCELL)), op=OP.mult)
            nc.vector.tensor_tensor(out=gate_v, in0=gate_v, in1=g2_v, op=OP.add)

            # ---- phase B: dense gate matmul (bf16) + LSTM math + combine ----
            # act layout per batch tile: [i(1024) | f(1024) | o(1024) | tanh(j)(1024)]
            # ncnh layout: [new_c(1024) | new_h(1024)]
            with tc.tile_pool(name="psB", bufs=2, space="PSUM") as psB:
                for t_ in range(NT):
                    act = work.tile([P, GC], F32, tag="act")
                    for half in range(2):
                        ps = psB.tile([P, 2048], F32, tag="mm")
                        for k in range(KT):
                            lhs = fTb_sb[k][:, t_ * P:(t_ + 1) * P]
                            for c4 in range(4):
                                col = half * 2048 + c4 * 512
                                nc.tensor.matmul(
                                    ps[:, c4 * 512:(c4 + 1) * 512],
                                    lhsT=lhs,
                                    rhs=wg_sb[k][:, col:col + 512],
                                    start=(k == 0), stop=(k == KT - 1))
                        if has_bg:
                            nc.vector.tensor_tensor(
                                out=ps[:], in0=ps[:],
                                in1=bg_sb[:, half * 2048:(half + 1) * 2048],
                                op=OP.add)
                        if half == 0:
                            nc.scalar.activation(act[:, 0:2048], ps[:], AF.Sigmoid)
                        else:
                            nc.scalar.activation(act[:, 2048:3072], ps[:, 0:1024],
                                                 AF.Sigmoid)
                            nc.scalar.activation(act[:, 3072:4096], ps[:, 1024:2048],
                                                 AF.Tanh)

                    # i*tanh(j) on GPSIMD (DVE is the scarcer engine)
                    tij = work.tile([P, NCELL * OUT], F32, tag="tij")
                    nc.gpsimd.tensor_tensor(out=tij[:], in0=act[:, 0:1024],
                                            in1=act[:, 3072:4096], op=OP.mult)
                    ncnh = work.tile([P, 2 * NCELL * OUT], F32, tag="ncnh")
                    c_bt = c_sb[:, t_ * OUT:(t_ + 1) * OUT]
                    nc.vector.tensor_tensor(
                        out=ncnh[:, 0:1024].rearrange("p (n o) -> p n o", o=OUT),
                        in0=act[:, 1024:2048].rearrange("p (n o) -> p n o", o=OUT),
                        in1=c_bt.unsqueeze(1).to_broadcast((P, NCELL, OUT)),
                        op=OP.mult)
                    nc.vector.tensor_tensor(out=ncnh[:, 0:1024],
                                            in0=ncnh[:, 0:1024], in1=tij[:],
                                            op=OP.add)
                    # tanh(new_c) overwrites the spent tanh(j) slot of act
                    thc = act[:, 3072:4096]
                    nc.scalar.activation(thc, ncnh[:, 0:1024], AF.Tanh)
                    nc.gpsimd.tensor_tensor(out=ncnh[:, 1024:2048],
                                            in0=act[:, 2048:3072], in1=thc,
                                            op=OP.mult)

                    # routed combine: both outputs per instruction via [P,2,256]
                    acc = work.tile([P, 2 * OUT], F32, tag="acc")
                    acc_v = acc[:].rearrange("p (u o) -> p u o", o=OUT)
                    src = ncnh[:].rearrange("p (u n o) -> p n u o", o=OUT, u=2)
                    nc.vector.tensor_scalar_mul(
                        acc_v, src[:, 0], gate[:, t_ * NCELL:t_ * NCELL + 1])
                    for n_ in range(1, NCELL):
                        nc.vector.scalar_tensor_tensor(
                            out=acc_v, in0=src[:, n_],
                            scalar=gate[:, t_ * NCELL + n_:t_ * NCELL + n_ + 1],
                            in1=acc_v, op0=OP.mult, op1=OP.add)
                    nc.sync.dma_start(out=nc_o[t_ * P:(t_ + 1) * P, :],
                                      in_=acc[:, 0:OUT])
                    nc.sync.dma_start(out=nh_o[t_ * P:(t_ + 1) * P, :],
                                      in_=acc[:, OUT:2 * OUT])
    nc.compile()
    return nc


_programs = {}


def _get_program(has_bg, has_bc):
    key = (has_bg, has_bc)
    if key not in _programs:
        _programs[key] = _build_program(has_bg, has_bc)
    return _programs[key]


def kernel(x, c, h, W_gates, b_gates, W_ctrl, b_ctrl):
    global LAST_RESULTS
    x = np.ascontiguousarray(np.asarray(x, dtype=np.float32))
    c = np.ascontiguousarray(np.asarray(c, dtype=np.float32))
    h = np.ascontiguousarray(np.asarray(h, dtype=np.float32))
    W_gates = np.asarray(W_gates, dtype=np.float32)
    b_gates = np.asarray(b_gates, dtype=np.float32)
    W_ctrl = np.ascontiguousarray(np.asarray(W_ctrl, dtype=np.float32))
    b_ctrl = np.asarray(b_ctrl, dtype=np.float32)

    featsT = np.ascontiguousarray(np.concatenate([x, h], axis=1).T)  # [D, B]
    # permute W_gates columns [d, n, g, o] -> gate-major [d, (i,f,o,j), n, o]
    wg_p = np.ascontiguousarray(
        W_gates.reshape(D, NCELL, 4, OUT)[:, :, [0, 2, 3, 1], :]
        .transpose(0, 2, 1, 3).reshape(D, GC))
    bg_p = np.ascontiguousarray(
        b_gates.reshape(NCELL, 4, OUT)[:, [0, 2, 3, 1], :]
        .transpose(1, 0, 2).reshape(1, GC))

    import ml_dtypes
    featsTb = featsT.astype(ml_dtypes.bfloat16)
    wg_b = wg_p.astype(ml_dtypes.bfloat16)

    has_bg = bool(np.any(b_gates))
    has_bc = bool(np.any(b_ctrl))
    prog = _get_program(has_bg, has_bc)

    in_maps = []
    for i in range(N_CORES):
        m = {
            "featsT": np.ascontiguousarray(featsT[:, i * BL:(i + 1) * BL]),
            "featsTb": np.ascontiguousarray(featsTb[:, i * BL:(i + 1) * BL]),
            "c_in": np.ascontiguousarray(c[i * BL:(i + 1) * BL]),
            "wgb": wg_b,
            "wc": W_ctrl,
        }
        if has_bg:
            m["bg"] = bg_p
        if has_bc:
            m["bc"] = np.ascontiguousarray(b_ctrl.reshape(1, NCELL))
        in_maps.append(m)

    res = run_bass_kernel_spmd(prog, in_maps, core_ids=list(range(N_CORES)),
                               trace=TRACE)
    LAST_RESULTS = res
    nh = np.concatenate([res.results[i]["nh_out"] for i in range(N_CORES)], axis=0)
    ncv = np.concatenate([res.results[i]["nc_out"] for i in range(N_CORES)], axis=0)
    return nh.astype(np.float32), ncv.astype(np.float32)


# revision 10
# speedup vs baseline: 1.0901x; 1.0901x over previous
"""Trainium2 Bass kernel for a top-2-of-4 routed LSTM cell bank (MoE routing).

Reference computation (per batch row b):
    feats    = concat(x[b], h[b])                      # [512]
    logits   = feats @ W_ctrl + b_ctrl                 # [4]
    gate     = top2_softmax(logits)                    # [4], 2 nonzero
    combined = feats @ W_gates + b_gates               # [4 cells, 4 gates, 256]
    i, j, f, o = gates;  new_c_n = sig(f)*c + sig(i)*tanh(j);  new_h_n = sig(o)*tanh(new_c_n)
    nh[b] = sum_n gate[n]*new_h_n ; nc[b] = sum_n gate[n]*new_c_n

Strategy: data-parallel over 8 NeuronCores (2048 batch rows each), weights
replicated.  Per core everything is dense and batch-tiled (16 tiles of 128
rows).  The routing logits are computed with true-fp32 matmuls (the smallest
top2/top3 logit gap in the dataset is ~2e-5, so reduced precision would flip
routing decisions); the big [2048,512]@[512,4096] gate matmul runs in bf16
(full PE stream rate; fp32 matmul is 4 cycles/row).  W_gates columns are
permuted host-side to gate-major [i|f|o|j] order so each activation function
covers one contiguous span per batch tile.  The heavy elementwise chain is
split across DVE and GPSIMD; new_c/new_h live in one tile so the routed
combine handles both outputs per instruction.
"""

import sys

for _p in ("/opt/trn_rl_repo", "/root/.axon_site/_ro/trn_rl_repo"):
    if _p not in sys.path:
        sys.path.append(_p)

import numpy as np

import concourse.bacc as bacc
from concourse import bass, mybir
from concourse.bass_utils import run_bass_kernel_spmd
from concourse.tile import TileContext

P = 128
N_CORES = 8
B = 16384
IN = 256
OUT = 256
NCELL = 4
D = IN + OUT          # 512
KT = D // P           # 4 contraction tiles
BL = B // N_CORES     # 2048 rows per core
NT = BL // P          # 16 batch tiles per core
GC = 4 * OUT * NCELL  # 4096 gate columns

F32 = mybir.dt.float32
BF16 = mybir.dt.bfloat16
U32 = mybir.dt.uint32
I32 = mybir.dt.int32
AF = mybir.ActivationFunctionType
OP = mybir.AluOpType

# test.py can flip these to capture a profiled run
TRACE = False
LAST_RESULTS = None


def _build_program(has_bg: bool, has_bc: bool):
    nc = bacc.Bacc("TRN2", target_bir_lowering=False, debug=False,
                   num_devices=N_CORES)

    featsT = nc.dram_tensor("featsT", (D, BL), F32, kind="ExternalInput").ap()
    featsTb = nc.dram_tensor("featsTb", (D, BL), BF16, kind="ExternalInput").ap()
    wgb = nc.dram_tensor("wgb", (D, GC), BF16, kind="ExternalInput").ap()
    c_in = nc.dram_tensor("c_in", (BL, OUT), F32, kind="ExternalInput").ap()
    wc = nc.dram_tensor("wc", (D, NCELL), F32, kind="ExternalInput").ap()
    bg = bc = None
    if has_bg:
        bg = nc.dram_tensor("bg", (1, GC), F32, kind="ExternalInput").ap()
    if has_bc:
        bc = nc.dram_tensor("bc", (1, NCELL), F32, kind="ExternalInput").ap()
    nh_o = nc.dram_tensor("nh_out", (BL, OUT), F32, kind="ExternalOutput").ap()
    nc_o = nc.dram_tensor("nc_out", (BL, OUT), F32, kind="ExternalOutput").ap()

    with TileContext(nc) as tc:
        with tc.tile_pool(name="const", bufs=1) as konst, \
             tc.tile_pool(name="work", bufs=2) as work:

            # ---- input loads; one HWDGE FIFO orders availability:
            # bf16 featsT -> W (half-major) -> c -> fp32 featsT (logits) ...
            fTb_sb = []
            for k in range(KT):
                t = konst.tile([P, BL], BF16, tag=f"fTb{k}", name=f"fTb_sb{k}")
                nc.sync.dma_start(
                    out=t[:], in_=featsTb.rearrange("(k p) b -> k p b", p=P)[k])
                fTb_sb.append(t)
            wg_sb = [konst.tile([P, GC], BF16, tag=f"wg{k}", name=f"wg_sb{k}")
                     for k in range(KT)]
            wg_dram = wgb.rearrange("(k p) n -> k p n", p=P)
            for half in range(2):
                for k in range(KT):
                    nc.sync.dma_start(
                        out=wg_sb[k][:, half * 2048:(half + 1) * 2048],
                        in_=wg_dram[k][:, half * 2048:(half + 1) * 2048])
            c_sb = konst.tile([P, NT * OUT], F32, tag="c")
            nc.gpsimd.dma_start(
                out=c_sb[:].rearrange("p (t o) -> p t o", o=OUT),
                in_=c_in.rearrange("(t p) o -> p t o", p=P))
            fT_sb = []
            for k in range(KT):
                t = konst.tile([P, BL], F32, tag=f"fT{k}")
                nc.sync.dma_start(
                    out=t[:], in_=featsT.rearrange("(k p) b -> k p b", p=P)[k])
                fT_sb.append(t)
            wc_sb = konst.tile([P, KT * NCELL], F32, tag="wc")
            nc.sync.dma_start(
                out=wc_sb[:].rearrange("p (k n) -> p k n", n=NCELL),
                in_=wc.rearrange("(k p) n -> p k n", p=P))
            bg_sb = bc_sb = None
            if has_bg:
                bg_sb = konst.tile([P, GC], F32, tag="bg")
                nc.sync.dma_start(out=bg_sb[:], in_=bg.partition_broadcast(P)[:, 0, :])
            if has_bc:
                bc_sb = konst.tile([P, NCELL], F32, tag="bc")
                nc.sync.dma_start(out=bc_sb[:], in_=bc.partition_broadcast(P)[:, 0, :])

            # gate-phase tiles (filled mid-loop, after bt1's matmuls)
            lg = konst.tile([P, NT * NCELL], F32, tag="lg")
            l8 = konst.tile([P, NT * 8], F32, tag="l8")
            mx8 = konst.tile([P, NT * 8], F32, tag="mx8")
            ix8 = konst.tile([P, NT * 8], U32, tag="ix8")
            diff = konst.tile([P, NT], F32, tag="diff")
            p1 = konst.tile([P, NT], F32, tag="p1")
            p2 = konst.tile([P, NT], F32, tag="p2")
            i1f = konst.tile([P, NT], F32, tag="i1f")
            i2f = konst.tile([P, NT], F32, tag="i2f")
            iota_i = konst.tile([P, NT * NCELL], I32, tag="iota_i")
            iota_f = konst.tile([P, NT * NCELL], F32, tag="iota_f")
            gate = konst.tile([P, NT * NCELL], F32, tag="gate")
            g2 = konst.tile([P, NT * NCELL], F32, tag="g2")

            def emit_logits_and_gates(lg_ps):
                # routing logits: true fp32 matmuls (reduced precision would
                # flip top-2 decisions; min top2/top3 gap here is ~2e-5)
                for t_ in range(NT):
                    for k in range(KT):
                        nc.tensor.matmul(
                            lg_ps[:, t_ * NCELL:(t_ + 1) * NCELL],
                            lhsT=fT_sb[k][:, t_ * P:(t_ + 1) * P],
                            rhs=wc_sb[:, k * NCELL:(k + 1) * NCELL],
                            start=(k == 0), stop=(k == KT - 1))
                nc.vector.tensor_copy(lg[:], lg_ps[:, 0:NT * NCELL])
                if has_bc:
                    nc.vector.tensor_tensor(
                        out=lg[:].rearrange("p (t n) -> p t n", n=NCELL),
                        in0=lg[:].rearrange("p (t n) -> p t n", n=NCELL),
                        in1=bc_sb[:].unsqueeze(1).to_broadcast((P, NT, NCELL)),
                        op=OP.add)
                nc.vector.memset(l8[:], -1e30)
                nc.vector.tensor_copy(
                    out=l8[:].rearrange("p (t e) -> p t e", e=8)[:, :, 0:NCELL],
                    in_=lg[:].rearrange("p (t n) -> p t n", n=NCELL))
                for t_ in range(NT):
                    nc.vector.max(mx8[:, t_ * 8:(t_ + 1) * 8],
                                  l8[:, t_ * 8:(t_ + 1) * 8])
                    nc.vector.max_index(ix8[:, t_ * 8:(t_ + 1) * 8],
                                        mx8[:, t_ * 8:(t_ + 1) * 8],
                                        l8[:, t_ * 8:(t_ + 1) * 8])
                mx_v = mx8[:].rearrange("p (t e) -> p t e", e=8)
                ix_v = ix8[:].rearrange("p (t e) -> p t e", e=8)
                nc.vector.tensor_tensor(out=diff[:].unsqueeze(2),
                                        in0=mx_v[:, :, 0:1], in1=mx_v[:, :, 1:2],
                                        op=OP.subtract)
                nc.scalar.activation(p1[:], diff[:], AF.Sigmoid)
                nc.vector.tensor_scalar(p2[:], p1[:], -1.0, 1.0, OP.mult, OP.add)
                nc.vector.tensor_copy(i1f[:].unsqueeze(2), ix_v[:, :, 0:1])
                nc.vector.tensor_copy(i2f[:].unsqueeze(2), ix_v[:, :, 1:2])
                nc.gpsimd.iota(iota_i[:], pattern=[[0, NT], [1, NCELL]],
                               base=0, channel_multiplier=0)
                nc.vector.tensor_copy(iota_f[:], iota_i[:])
                iota_v = iota_f[:].rearrange("p (t n) -> p t n", n=NCELL)
                gate_v = gate[:].rearrange("p (t n) -> p t n", n=NCELL)
                g2_v = g2[:].rearrange("p (t n) -> p t n", n=NCELL)
                nc.vector.tensor_tensor(
                    out=gate_v,
                    in0=i1f[:].unsqueeze(2).to_broadcast((P, NT, NCELL)),
                    in1=iota_v, op=OP.is_equal)
                nc.vector.tensor_tensor(
                    out=gate_v, in0=gate_v,
                    in1=p1[:].unsqueeze(2).to_broadcast((P, NT, NCELL)), op=OP.mult)
                nc.vector.tensor_tensor(
                    out=g2_v,
                    in0=i2f[:].unsqueeze(2).to_broadcast((P, NT, NCELL)),
                    in1=iota_v, op=OP.is_equal)
                nc.vector.tensor_tensor(
                    out=g2_v, in0=g2_v,
                    in1=p2[:].unsqueeze(2).to_broadcast((P, NT, NCELL)), op=OP.mult)
                nc.vector.tensor_tensor(out=gate_v, in0=gate_v, in1=g2_v, op=OP.add)

            # ---- phase B: dense gate matmul (bf16) + LSTM math + combine ----
            # act layout per batch tile: [i(1024) | f(1024) | o(1024) | tanh(j)(1024)]
            # (tanh(j) slot is later overwritten with tanh(new_c));
            # ncnh layout: [new_c(1024) | new_h(1024)]
            # Engine streams are in-order, so thc/new_h run 1 tile behind the
            # matmuls and the routed combine 2 tiles behind; the gate chain is
            # emitted after bt1 so nothing ever waits on it.
            acts = [None] * NT
            ncnhs = [None] * NT

            def emit_thc_newh(j_):
                thc = acts[j_][:, 3072:4096]
                nc.scalar.activation(thc, ncnhs[j_][:, 0:1024], AF.Tanh)
                nc.gpsimd.tensor_tensor(out=ncnhs[j_][:, 1024:2048],
                                        in0=acts[j_][:, 2048:3072], in1=thc,
                                        op=OP.mult)

            def emit_combine(j_):
                acc = work.tile([P, 2 * OUT], F32, tag="acc", name=f"acc{j_}",
                                bufs=3)
                acc_v = acc[:].rearrange("p (u o) -> p u o", o=OUT)
                src = ncnhs[j_][:].rearrange("p (u n o) -> p n u o", o=OUT, u=2)
                nc.vector.tensor_scalar(
                    acc_v, src[:, 0], gate[:, j_ * NCELL:j_ * NCELL + 1],
                    None, OP.mult)
                for n_ in range(1, NCELL):
                    nc.vector.scalar_tensor_tensor(
                        out=acc_v, in0=src[:, n_],
                        scalar=gate[:, j_ * NCELL + n_:j_ * NCELL + n_ + 1],
                        in1=acc_v, op0=OP.mult, op1=OP.add)
                nc.sync.dma_start(out=nc_o[j_ * P:(j_ + 1) * P, :],
                                  in_=acc[:, 0:OUT])
                nc.sync.dma_start(out=nh_o[j_ * P:(j_ + 1) * P, :],
                                  in_=acc[:, OUT:2 * OUT])

            with tc.tile_pool(name="psB", bufs=2, space="PSUM") as psB:
                for t_ in range(NT):
                    act = work.tile([P, GC], F32, tag="act", name=f"act{t_}",
                                    bufs=2)
                    acts[t_] = act
                    for half in range(2):
                        ps = psB.tile([P, 2048], F32, tag="mm", name=f"mm{t_}_{half}")
                        for k in range(KT):
                            lhs = fTb_sb[k][:, t_ * P:(t_ + 1) * P]
                            for c4 in range(4):
                                col = half * 2048 + c4 * 512
                                nc.tensor.matmul(
                                    ps[:, c4 * 512:(c4 + 1) * 512],
                                    lhsT=lhs,
                                    rhs=wg_sb[k][:, col:col + 512],
                                    start=(k == 0), stop=(k == KT - 1))
                        if has_bg:
                            nc.vector.tensor_tensor(
                                out=ps[:], in0=ps[:],
                                in1=bg_sb[:, half * 2048:(half + 1) * 2048],
                                op=OP.add)
                        if half == 0:
                            nc.scalar.activation(act[:, 0:2048], ps[:], AF.Sigmoid)
                        else:
                            nc.scalar.activation(act[:, 2048:3072], ps[:, 0:1024],
                                                 AF.Sigmoid)
                            nc.scalar.activation(act[:, 3072:4096], ps[:, 1024:2048],
                                                 AF.Tanh)
                    if t_ >= 1:
                        emit_thc_newh(t_ - 1)

                    tij = work.tile([P, NCELL * OUT], F32, tag="tij",
                                    name=f"tij{t_}")
                    nc.vector.tensor_tensor(out=tij[:], in0=act[:, 0:1024],
                                            in1=act[:, 3072:4096], op=OP.mult)
                    ncnh = work.tile([P, 2 * NCELL * OUT], F32, tag="ncnh",
                                     name=f"ncnh{t_}", bufs=5)
                    ncnhs[t_] = ncnh
                    c_bt = c_sb[:, t_ * OUT:(t_ + 1) * OUT]
                    nc.vector.tensor_tensor(
                        out=ncnh[:, 0:1024].rearrange("p (n o) -> p n o", o=OUT),
                        in0=act[:, 1024:2048].rearrange("p (n o) -> p n o", o=OUT),
                        in1=c_bt.unsqueeze(1).to_broadcast((P, NCELL, OUT)),
                        op=OP.mult)
                    nc.vector.tensor_tensor(out=ncnh[:, 0:1024],
                                            in0=ncnh[:, 0:1024], in1=tij[:],
                                            op=OP.add)
                    if t_ == 2:
                        lg_ps = psB.tile([P, 2048], F32, tag="mm", name="mm_lg")
                        emit_logits_and_gates(lg_ps)
                    if t_ >= 4:
                        emit_combine(t_ - 4)

                emit_thc_newh(NT - 1)
                for j_ in range(NT - 4, NT):
                    emit_combine(j_)
    nc.compile()
    return nc


_programs = {}


def _get_program(has_bg, has_bc):
    key = (has_bg, has_bc)
    if key not in _programs:
        _programs[key] = _build_program(has_bg, has_bc)
    return _programs[key]


def kernel(x, c, h, W_gates, b_gates, W_ctrl, b_ctrl):
    global LAST_RESULTS
    x = np.ascontiguousarray(np.asarray(x, dtype=np.float32))
    c = np.ascontiguousarray(np.asarray(c, dtype=np.float32))
    h = np.ascontiguousarray(np.asarray(h, dtype=np.float32))
    W_gates = np.asarray(W_gates, dtype=np.float32)
    b_gates = np.asarray(b_gates, dtype=np.float32)
    W_ctrl = np.ascontiguousarray(np.asarray(W_ctrl, dtype=np.float32))
    b_ctrl = np.asarray(b_ctrl, dtype=np.float32)

    featsT = np.ascontiguousarray(np.concatenate([x, h], axis=1).T)  # [D, B]
    # permute W_gates columns [d, n, g, o] -> gate-major [d, (i,f,o,j), n, o]
    wg_p = np.ascontiguousarray(
        W_gates.reshape(D, NCELL, 4, OUT)[:, :, [0, 2, 3, 1], :]
        .transpose(0, 2, 1, 3).reshape(D, GC))
    bg_p = np.ascontiguousarray(
        b_gates.reshape(NCELL, 4, OUT)[:, [0, 2, 3, 1], :]
        .transpose(1, 0, 2).reshape(1, GC))

    import ml_dtypes
    featsTb = featsT.astype(ml_dtypes.bfloat16)
    wg_b = wg_p.astype(ml_dtypes.bfloat16)

    has_bg = bool(np.any(b_gates))
    has_bc = bool(np.any(b_ctrl))
    prog = _get_program(has_bg, has_bc)

    in_maps = []
    for i in range(N_CORES):
        m = {
            "featsT": np.ascontiguousarray(featsT[:, i * BL:(i + 1) * BL]),
            "featsTb": np.ascontiguousarray(featsTb[:, i * BL:(i + 1) * BL]),
            "c_in": np.ascontiguousarray(c[i * BL:(i + 1) * BL]),
            "wgb": wg_b,
            "wc": W_ctrl,
        }
        if has_bg:
            m["bg"] = bg_p
        if has_bc:
            m["bc"] = np.ascontiguousarray(b_ctrl.reshape(1, NCELL))
        in_maps.append(m)

    res = run_bass_kernel_spmd(prog, in_maps, core_ids=list(range(N_CORES)),
                               trace=TRACE)
    LAST_RESULTS = res
    nh = np.concatenate([res.results[i]["nh_out"] for i in range(N_CORES)], axis=0)
    ncv = np.concatenate([res.results[i]["nc_out"] for i in range(N_CORES)], axis=0)
    return nh.astype(np.float32), ncv.astype(np.float32)


# revision 11
# speedup vs baseline: 1.2101x; 1.1101x over previous
"""Trainium2 Bass kernel for a top-2-of-4 routed LSTM cell bank (MoE routing).

Reference computation (per batch row b):
    feats    = concat(x[b], h[b])                      # [512]
    logits   = feats @ W_ctrl + b_ctrl                 # [4]
    gate     = top2_softmax(logits)                    # [4], 2 nonzero
    combined = feats @ W_gates + b_gates               # [4 cells, 4 gates, 256]
    i, j, f, o = gates;  new_c_n = sig(f)*c + sig(i)*tanh(j);  new_h_n = sig(o)*tanh(new_c_n)
    nh[b] = sum_n gate[n]*new_h_n ; nc[b] = sum_n gate[n]*new_c_n

Strategy: data-parallel over 8 NeuronCores (2048 batch rows each), weights
replicated.  Per core everything is dense and batch-tiled (16 tiles of 128
rows).  The routing logits are computed with true-fp32 matmuls (the smallest
top2/top3 logit gap in the dataset is ~2e-5, so reduced precision would flip
routing decisions); the big [2048,512]@[512,4096] gate matmul runs in bf16
(full PE stream rate; fp32 matmul is 4 cycles/row).  W_gates columns are
permuted host-side to gate-major [i|f|o|j] order so each activation function
covers one contiguous span per batch tile.  The heavy elementwise chain is
split across DVE and GPSIMD; new_c/new_h live in one tile so the routed
combine handles both outputs per instruction.
"""

import sys

for _p in ("/opt/trn_rl_repo", "/root/.axon_site/_ro/trn_rl_repo"):
    if _p not in sys.path:
        sys.path.append(_p)

import numpy as np

import concourse.bacc as bacc
from concourse import bass, mybir
from concourse.bass_utils import run_bass_kernel_spmd
from concourse.tile import TileContext

P = 128
N_CORES = 8
B = 16384
IN = 256
OUT = 256
NCELL = 4
D = IN + OUT          # 512
KT = D // P           # 4 contraction tiles
BL = B // N_CORES     # 2048 rows per core
NT = BL // P          # 16 batch tiles per core
GC = 4 * OUT * NCELL  # 4096 gate columns

F32 = mybir.dt.float32
BF16 = mybir.dt.bfloat16
U32 = mybir.dt.uint32
I32 = mybir.dt.int32
AF = mybir.ActivationFunctionType
OP = mybir.AluOpType

# test.py can flip these to capture a profiled run
TRACE = False
LAST_RESULTS = None


def _build_program(has_bg: bool, has_bc: bool):
    nc = bacc.Bacc("TRN2", target_bir_lowering=False, debug=False,
                   num_devices=N_CORES)

    featsT = nc.dram_tensor("featsT", (D, BL), F32, kind="ExternalInput").ap()
    featsTb = nc.dram_tensor("featsTb", (D, BL), BF16, kind="ExternalInput").ap()
    wgb = nc.dram_tensor("wgb", (D, GC), BF16, kind="ExternalInput").ap()
    c_in = nc.dram_tensor("c_in", (BL, OUT), F32, kind="ExternalInput").ap()
    wc = nc.dram_tensor("wc", (D, NCELL), F32, kind="ExternalInput").ap()
    bg = bc = None
    if has_bg:
        bg = nc.dram_tensor("bg", (1, GC), F32, kind="ExternalInput").ap()
    if has_bc:
        bc = nc.dram_tensor("bc", (1, NCELL), F32, kind="ExternalInput").ap()
    nh_o = nc.dram_tensor("nh_out", (BL, OUT), F32, kind="ExternalOutput").ap()
    nc_o = nc.dram_tensor("nc_out", (BL, OUT), F32, kind="ExternalOutput").ap()

    with TileContext(nc) as tc:
        with tc.tile_pool(name="const", bufs=1) as konst, \
             tc.tile_pool(name="work", bufs=2) as work:

            # ---- input loads; one HWDGE FIFO orders availability:
            # bf16 featsT -> W (half-major) -> c -> fp32 featsT (logits) ...
            fTb_sb = []
            for k in range(KT):
                t = konst.tile([P, BL], BF16, tag=f"fTb{k}", name=f"fTb_sb{k}")
                nc.sync.dma_start(
                    out=t[:], in_=featsTb.rearrange("(k p) b -> k p b", p=P)[k])
                fTb_sb.append(t)
            wg_sb = [konst.tile([P, GC], BF16, tag=f"wg{k}", name=f"wg_sb{k}")
                     for k in range(KT)]
            wg_dram = wgb.rearrange("(k p) n -> k p n", p=P)
            for half in range(2):
                for k in range(KT):
                    nc.sync.dma_start(
                        out=wg_sb[k][:, half * 2048:(half + 1) * 2048],
                        in_=wg_dram[k][:, half * 2048:(half + 1) * 2048])
            c_sb = konst.tile([P, NT * OUT], F32, tag="c")
            nc.gpsimd.dma_start(
                out=c_sb[:].rearrange("p (t o) -> p t o", o=OUT),
                in_=c_in.rearrange("(t p) o -> p t o", p=P))
            fT_sb = []
            for k in range(KT):
                t = konst.tile([P, BL], F32, tag=f"fT{k}")
                nc.sync.dma_start(
                    out=t[:], in_=featsT.rearrange("(k p) b -> k p b", p=P)[k])
                fT_sb.append(t)
            wc_sb = konst.tile([P, KT * NCELL], F32, tag="wc")
            nc.sync.dma_start(
                out=wc_sb[:].rearrange("p (k n) -> p k n", n=NCELL),
                in_=wc.rearrange("(k p) n -> p k n", p=P))
            bg_sb = bc_sb = None
            if has_bg:
                bg_sb = konst.tile([P, GC], F32, tag="bg")
                nc.sync.dma_start(out=bg_sb[:], in_=bg.partition_broadcast(P)[:, 0, :])
            if has_bc:
                bc_sb = konst.tile([P, NCELL], F32, tag="bc")
                nc.sync.dma_start(out=bc_sb[:], in_=bc.partition_broadcast(P)[:, 0, :])

            # gate-phase tiles (filled mid-loop, after bt1's matmuls)
            lg = konst.tile([P, NT * NCELL], F32, tag="lg")
            l8 = konst.tile([P, NT * 8], F32, tag="l8")
            mx8 = konst.tile([P, NT * 8], F32, tag="mx8")
            ix8 = konst.tile([P, NT * 8], U32, tag="ix8")
            diff = konst.tile([P, NT], F32, tag="diff")
            p1 = konst.tile([P, NT], F32, tag="p1")
            p2 = konst.tile([P, NT], F32, tag="p2")
            i1f = konst.tile([P, NT], F32, tag="i1f")
            i2f = konst.tile([P, NT], F32, tag="i2f")
            iota_i = konst.tile([P, NT * NCELL], I32, tag="iota_i")
            iota_f = konst.tile([P, NT * NCELL], F32, tag="iota_f")
            gate = konst.tile([P, NT * NCELL], F32, tag="gate")
            g2 = konst.tile([P, NT * NCELL], F32, tag="g2")

            def emit_logits_and_gates(lg_ps):
                # routing logits: true fp32 matmuls (reduced precision would
                # flip top-2 decisions; min top2/top3 gap here is ~2e-5)
                for t_ in range(NT):
                    for k in range(KT):
                        nc.tensor.matmul(
                            lg_ps[:, t_ * NCELL:(t_ + 1) * NCELL],
                            lhsT=fT_sb[k][:, t_ * P:(t_ + 1) * P],
                            rhs=wc_sb[:, k * NCELL:(k + 1) * NCELL],
                            start=(k == 0), stop=(k == KT - 1))
                nc.vector.tensor_copy(lg[:], lg_ps[:, 0:NT * NCELL])
                if has_bc:
                    nc.vector.tensor_tensor(
                        out=lg[:].rearrange("p (t n) -> p t n", n=NCELL),
                        in0=lg[:].rearrange("p (t n) -> p t n", n=NCELL),
                        in1=bc_sb[:].unsqueeze(1).to_broadcast((P, NT, NCELL)),
                        op=OP.add)
                nc.vector.memset(l8[:], -1e30)
                nc.vector.tensor_copy(
                    out=l8[:].rearrange("p (t e) -> p t e", e=8)[:, :, 0:NCELL],
                    in_=lg[:].rearrange("p (t n) -> p t n", n=NCELL))
                for t_ in range(NT):
                    nc.vector.max(mx8[:, t_ * 8:(t_ + 1) * 8],
                                  l8[:, t_ * 8:(t_ + 1) * 8])
                    nc.vector.max_index(ix8[:, t_ * 8:(t_ + 1) * 8],
                                        mx8[:, t_ * 8:(t_ + 1) * 8],
                                        l8[:, t_ * 8:(t_ + 1) * 8])
                mx_v = mx8[:].rearrange("p (t e) -> p t e", e=8)
                ix_v = ix8[:].rearrange("p (t e) -> p t e", e=8)
                nc.vector.tensor_tensor(out=diff[:].unsqueeze(2),
                                        in0=mx_v[:, :, 0:1], in1=mx_v[:, :, 1:2],
                                        op=OP.subtract)
                nc.scalar.activation(p1[:], diff[:], AF.Sigmoid)
                nc.vector.tensor_scalar(p2[:], p1[:], -1.0, 1.0, OP.mult, OP.add)
                nc.vector.tensor_copy(i1f[:].unsqueeze(2), ix_v[:, :, 0:1])
                nc.vector.tensor_copy(i2f[:].unsqueeze(2), ix_v[:, :, 1:2])
                nc.gpsimd.iota(iota_i[:], pattern=[[0, NT], [1, NCELL]],
                               base=0, channel_multiplier=0)
                nc.vector.tensor_copy(iota_f[:], iota_i[:])
                iota_v = iota_f[:].rearrange("p (t n) -> p t n", n=NCELL)
                gate_v = gate[:].rearrange("p (t n) -> p t n", n=NCELL)
                g2_v = g2[:].rearrange("p (t n) -> p t n", n=NCELL)
                nc.vector.tensor_tensor(
                    out=gate_v,
                    in0=i1f[:].unsqueeze(2).to_broadcast((P, NT, NCELL)),
                    in1=iota_v, op=OP.is_equal)
                nc.vector.tensor_tensor(
                    out=gate_v, in0=gate_v,
                    in1=p1[:].unsqueeze(2).to_broadcast((P, NT, NCELL)), op=OP.mult)
                nc.vector.tensor_tensor(
                    out=g2_v,
                    in0=i2f[:].unsqueeze(2).to_broadcast((P, NT, NCELL)),
                    in1=iota_v, op=OP.is_equal)
                nc.vector.tensor_tensor(
                    out=g2_v, in0=g2_v,
                    in1=p2[:].unsqueeze(2).to_broadcast((P, NT, NCELL)), op=OP.mult)
                nc.vector.tensor_tensor(out=gate_v, in0=gate_v, in1=g2_v, op=OP.add)

            # ---- phase B: dense gate matmul (bf16) + LSTM math + combine ----
            # act layout per batch tile: [i(1024) | f(1024) | o(1024) | tanh(j)(1024)]
            # (tanh(j) slot is later overwritten with tanh(new_c));
            # ncnh layout: [new_c(1024) | new_h(1024)]
            # Engine streams are in-order, so thc/new_h run 1 tile behind the
            # matmuls and the routed combine 2 tiles behind; the gate chain is
            # emitted after bt1 so nothing ever waits on it.
            acts = [None] * NT
            ncnhs = [None] * NT

            def emit_thc_newh(j_):
                thc = acts[j_][:, 3072:4096]
                nc.scalar.activation(thc, ncnhs[j_][:, 0:1024], AF.Tanh)
                nc.vector.tensor_tensor(out=ncnhs[j_][:, 1024:2048],
                                        in0=acts[j_][:, 2048:3072], in1=thc,
                                        op=OP.mult)

            def emit_combine(j_):
                acc = work.tile([P, 2 * OUT], F32, tag="acc", name=f"acc{j_}",
                                bufs=3)
                acc_v = acc[:].rearrange("p (u o) -> p u o", o=OUT)
                src = ncnhs[j_][:].rearrange("p (u n o) -> p n u o", o=OUT, u=2)
                nc.vector.tensor_scalar(
                    acc_v, src[:, 0], gate[:, j_ * NCELL:j_ * NCELL + 1],
                    None, OP.mult)
                for n_ in range(1, NCELL):
                    nc.vector.scalar_tensor_tensor(
                        out=acc_v, in0=src[:, n_],
                        scalar=gate[:, j_ * NCELL + n_:j_ * NCELL + n_ + 1],
                        in1=acc_v, op0=OP.mult, op1=OP.add)
                nc.sync.dma_start(out=nc_o[j_ * P:(j_ + 1) * P, :],
                                  in_=acc[:, 0:OUT])
                nc.sync.dma_start(out=nh_o[j_ * P:(j_ + 1) * P, :],
                                  in_=acc[:, OUT:2 * OUT])

            with tc.tile_pool(name="psB", bufs=2, space="PSUM") as psB:
                for t_ in range(NT):
                    act = work.tile([P, GC], F32, tag="act", name=f"act{t_}",
                                    bufs=2)
                    acts[t_] = act
                    for half in range(2):
                        ps = psB.tile([P, 2048], F32, tag="mm", name=f"mm{t_}_{half}")
                        for k in range(KT):
                            lhs = fTb_sb[k][:, t_ * P:(t_ + 1) * P]
                            for c4 in range(4):
                                col = half * 2048 + c4 * 512
                                nc.tensor.matmul(
                                    ps[:, c4 * 512:(c4 + 1) * 512],
                                    lhsT=lhs,
                                    rhs=wg_sb[k][:, col:col + 512],
                                    start=(k == 0), stop=(k == KT - 1))
                        if has_bg:
                            nc.vector.tensor_tensor(
                                out=ps[:], in0=ps[:],
                                in1=bg_sb[:, half * 2048:(half + 1) * 2048],
                                op=OP.add)
                        if half == 0:
                            nc.scalar.activation(act[:, 0:2048], ps[:], AF.Sigmoid)
                        else:
                            nc.scalar.activation(act[:, 2048:3072], ps[:, 0:1024],
                                                 AF.Sigmoid)
                            nc.scalar.activation(act[:, 3072:4096], ps[:, 1024:2048],
                                                 AF.Tanh)
                    if t_ >= 1:
                        emit_thc_newh(t_ - 1)

                    tij = work.tile([P, NCELL * OUT], F32, tag="tij",
                                    name=f"tij{t_}")
                    nc.vector.tensor_tensor(out=tij[:], in0=act[:, 0:1024],
                                            in1=act[:, 3072:4096], op=OP.mult)
                    ncnh = work.tile([P, 2 * NCELL * OUT], F32, tag="ncnh",
                                     name=f"ncnh{t_}", bufs=5)
                    ncnhs[t_] = ncnh
                    c_bt = c_sb[:, t_ * OUT:(t_ + 1) * OUT]
                    nc.vector.tensor_tensor(
                        out=ncnh[:, 0:1024].rearrange("p (n o) -> p n o", o=OUT),
                        in0=act[:, 1024:2048].rearrange("p (n o) -> p n o", o=OUT),
                        in1=c_bt.unsqueeze(1).to_broadcast((P, NCELL, OUT)),
                        op=OP.mult)
                    nc.vector.tensor_tensor(out=ncnh[:, 0:1024],
                                            in0=ncnh[:, 0:1024], in1=tij[:],
                                            op=OP.add)
                    if t_ == 3:
                        lg_ps = psB.tile([P, 2048], F32, tag="mm", name="mm_lg")
                        emit_logits_and_gates(lg_ps)
                    if t_ >= 4:
                        emit_combine(t_ - 4)

                emit_thc_newh(NT - 1)
                for j_ in range(NT - 4, NT):
                    emit_combine(j_)
    nc.compile()
    return nc


_programs = {}


def _get_program(has_bg, has_bc):
    key = (has_bg, has_bc)
    if key not in _programs:
        _programs[key] = _build_program(has_bg, has_bc)
    return _programs[key]


def kernel(x, c, h, W_gates, b_gates, W_ctrl, b_ctrl):
    global LAST_RESULTS
    x = np.ascontiguousarray(np.asarray(x, dtype=np.float32))
    c = np.ascontiguousarray(np.asarray(c, dtype=np.float32))
    h = np.ascontiguousarray(np.asarray(h, dtype=np.float32))
    W_gates = np.asarray(W_gates, dtype=np.float32)
    b_gates = np.asarray(b_gates, dtype=np.float32)
    W_ctrl = np.ascontiguousarray(np.asarray(W_ctrl, dtype=np.float32))
    b_ctrl = np.asarray(b_ctrl, dtype=np.float32)

    featsT = np.ascontiguousarray(np.concatenate([x, h], axis=1).T)  # [D, B]
    # permute W_gates columns [d, n, g, o] -> gate-major [d, (i,f,o,j), n, o]
    wg_p = np.ascontiguousarray(
        W_gates.reshape(D, NCELL, 4, OUT)[:, :, [0, 2, 3, 1], :]
        .transpose(0, 2, 1, 3).reshape(D, GC))
    bg_p = np.ascontiguousarray(
        b_gates.reshape(NCELL, 4, OUT)[:, [0, 2, 3, 1], :]
        .transpose(1, 0, 2).reshape(1, GC))

    import ml_dtypes
    featsTb = featsT.astype(ml_dtypes.bfloat16)
    wg_b = wg_p.astype(ml_dtypes.bfloat16)

    has_bg = bool(np.any(b_gates))
    has_bc = bool(np.any(b_ctrl))
    prog = _get_program(has_bg, has_bc)

    in_maps = []
    for i in range(N_CORES):
        m = {
            "featsT": np.ascontiguousarray(featsT[:, i * BL:(i + 1) * BL]),
            "featsTb": np.ascontiguousarray(featsTb[:, i * BL:(i + 1) * BL]),
            "c_in": np.ascontiguousarray(c[i * BL:(i + 1) * BL]),
            "wgb": wg_b,
            "wc": W_ctrl,
        }
        if has_bg:
            m["bg"] = bg_p
        if has_bc:
            m["bc"] = np.ascontiguousarray(b_ctrl.reshape(1, NCELL))
        in_maps.append(m)

    res = run_bass_kernel_spmd(prog, in_maps, core_ids=list(range(N_CORES)),
                               trace=TRACE)
    LAST_RESULTS = res
    nh = np.concatenate([res.results[i]["nh_out"] for i in range(N_CORES)], axis=0)
    ncv = np.concatenate([res.results[i]["nc_out"] for i in range(N_CORES)], axis=0)
    return nh.astype(np.float32), ncv.astype(np.float32)


# revision 13
# speedup vs baseline: 1.3458x; 1.1122x over previous
"""Trainium2 Bass kernel for a top-2-of-4 routed LSTM cell bank (MoE routing).

Reference computation (per batch row b):
    feats    = concat(x[b], h[b])                      # [512]
    logits   = feats @ W_ctrl + b_ctrl                 # [4]
    gate     = top2_softmax(logits)                    # [4], 2 nonzero
    combined = feats @ W_gates + b_gates               # [4 cells, 4 gates, 256]
    i, j, f, o = gates;  new_c_n = sig(f)*c + sig(i)*tanh(j);  new_h_n = sig(o)*tanh(new_c_n)
    nh[b] = sum_n gate[n]*new_h_n ; nc[b] = sum_n gate[n]*new_c_n

Strategy: data-parallel over 8 NeuronCores (2048 batch rows each), weights
replicated.  Per core everything is dense and batch-tiled (16 tiles of 128
rows).  The routing logits are computed with true-fp32 matmuls (the smallest
top2/top3 logit gap in the dataset is ~2e-5, so reduced precision would flip
routing decisions); the big [2048,512]@[512,4096] gate matmul runs in bf16
(full PE stream rate; fp32 matmul is 4 cycles/row).  W_gates columns are
permuted host-side to gate-major [i|f|o|j] order so each activation function
covers one contiguous span per batch tile.  The heavy elementwise chain is
split across DVE and GPSIMD; new_c/new_h live in one tile so the routed
combine handles both outputs per instruction.
"""

import sys

for _p in ("/opt/trn_rl_repo", "/root/.axon_site/_ro/trn_rl_repo"):
    if _p not in sys.path:
        sys.path.append(_p)

import numpy as np

import concourse.bacc as bacc
from concourse import bass, mybir
from concourse.bass_utils import run_bass_kernel_spmd
from concourse.tile import TileContext

P = 128
N_CORES = 8
B = 16384
IN = 256
OUT = 256
NCELL = 4
D = IN + OUT          # 512
KT = D // P           # 4 contraction tiles
BL = B // N_CORES     # 2048 rows per core
NT = BL // P          # 16 batch tiles per core
GC = 4 * OUT * NCELL  # 4096 gate columns

F32 = mybir.dt.float32
BF16 = mybir.dt.bfloat16
U32 = mybir.dt.uint32
I32 = mybir.dt.int32
AF = mybir.ActivationFunctionType
OP = mybir.AluOpType

# elementwise intermediates in bf16: 2x DVE throughput, half the SBUF
EW_BF16 = True

# test.py can flip these to capture a profiled run
TRACE = False
LAST_RESULTS = None


def _build_program(has_bg: bool, has_bc: bool):
    nc = bacc.Bacc("TRN2", target_bir_lowering=False, debug=False,
                   num_devices=N_CORES)

    featsT = nc.dram_tensor("featsT", (D, BL), F32, kind="ExternalInput").ap()
    featsTb = nc.dram_tensor("featsTb", (D, BL), BF16, kind="ExternalInput").ap()
    wgb = nc.dram_tensor("wgb", (D, GC), BF16, kind="ExternalInput").ap()
    c_in = nc.dram_tensor("c_in", (BL, OUT), BF16 if EW_BF16 else F32,
                          kind="ExternalInput").ap()
    wc = nc.dram_tensor("wc", (D, NCELL), F32, kind="ExternalInput").ap()
    bg = bc = None
    if has_bg:
        bg = nc.dram_tensor("bg", (1, GC), F32, kind="ExternalInput").ap()
    if has_bc:
        bc = nc.dram_tensor("bc", (1, NCELL), F32, kind="ExternalInput").ap()
    nh_o = nc.dram_tensor("nh_out", (BL, OUT), F32, kind="ExternalOutput").ap()
    nc_o = nc.dram_tensor("nc_out", (BL, OUT), F32, kind="ExternalOutput").ap()

    with TileContext(nc) as tc:
        with tc.tile_pool(name="const", bufs=1) as konst, \
             tc.tile_pool(name="work", bufs=2) as work:

            # ---- input loads; one HWDGE FIFO gives strict priority order:
            # bf16 featsT -> W halves -> c -> fp32 featsT (logits late is fine)
            EW = BF16 if EW_BF16 else F32
            fTb_sb = konst.tile([P, KT * BL], BF16, tag="fTb")
            nc.sync.dma_start(
                out=fTb_sb[:].rearrange("p (k b) -> p k b", b=BL),
                in_=featsTb.rearrange("(k p) b -> p k b", p=P))
            wg_sb = konst.tile([P, KT * GC], BF16, tag="wg")
            wg_v = wg_sb[:].rearrange("p (k n) -> p k n", n=GC)
            wg_src = wgb.rearrange("(k p) n -> p k n", p=P)
            for half in range(2):
                for kp in range(2):
                    nc.sync.dma_start(
                        out=wg_v[:, 2 * kp:2 * kp + 2, half * 2048:(half + 1) * 2048],
                        in_=wg_src[:, 2 * kp:2 * kp + 2, half * 2048:(half + 1) * 2048])
            c_sb = konst.tile([P, NT * OUT], EW, tag="c")
            nc.sync.dma_start(
                out=c_sb[:].rearrange("p (t o) -> p t o", o=OUT),
                in_=c_in.rearrange("(t p) o -> p t o", p=P))
            fT_sb = konst.tile([P, KT * BL], F32, tag="fT")
            nc.sync.dma_start(
                out=fT_sb[:].rearrange("p (k b) -> p k b", b=BL),
                in_=featsT.rearrange("(k p) b -> p k b", p=P))
            wc_sb = konst.tile([P, KT * NCELL], F32, tag="wc")
            nc.sync.dma_start(
                out=wc_sb[:].rearrange("p (k n) -> p k n", n=NCELL),
                in_=wc.rearrange("(k p) n -> p k n", p=P))
            bg_sb = bc_sb = None
            if has_bg:
                bg_sb = konst.tile([P, GC], F32, tag="bg")
                nc.sync.dma_start(out=bg_sb[:], in_=bg.partition_broadcast(P)[:, 0, :])
            if has_bc:
                bc_sb = konst.tile([P, NCELL], F32, tag="bc")
                nc.sync.dma_start(out=bc_sb[:], in_=bc.partition_broadcast(P)[:, 0, :])

            # gate-phase tiles (filled mid-loop, after bt1's matmuls)
            lg = konst.tile([P, NT * NCELL], F32, tag="lg")
            l8 = konst.tile([P, NT * 8], F32, tag="l8")
            mx8 = konst.tile([P, NT * 8], F32, tag="mx8")
            ix8 = konst.tile([P, NT * 8], U32, tag="ix8")
            diff = konst.tile([P, NT], F32, tag="diff")
            p1 = konst.tile([P, NT], F32, tag="p1")
            p2 = konst.tile([P, NT], F32, tag="p2")
            i1f = konst.tile([P, NT], F32, tag="i1f")
            i2f = konst.tile([P, NT], F32, tag="i2f")
            iota_i = konst.tile([P, NT * NCELL], I32, tag="iota_i")
            iota_f = konst.tile([P, NT * NCELL], F32, tag="iota_f")
            gate = konst.tile([P, NT * NCELL], F32, tag="gate")
            g2 = konst.tile([P, NT * NCELL], F32, tag="g2")

            def emit_logits_and_gates(lg_ps):
                # routing logits: true fp32 matmuls (reduced precision would
                # flip top-2 decisions; min top2/top3 gap here is ~2e-5)
                for t_ in range(NT):
                    for k in range(KT):
                        nc.tensor.matmul(
                            lg_ps[:, t_ * NCELL:(t_ + 1) * NCELL],
                            lhsT=fT_sb[:, k * BL + t_ * P:k * BL + (t_ + 1) * P],
                            rhs=wc_sb[:, k * NCELL:(k + 1) * NCELL],
                            start=(k == 0), stop=(k == KT - 1))
                nc.vector.tensor_copy(lg[:], lg_ps[:, 0:NT * NCELL])
                if has_bc:
                    nc.vector.tensor_tensor(
                        out=lg[:].rearrange("p (t n) -> p t n", n=NCELL),
                        in0=lg[:].rearrange("p (t n) -> p t n", n=NCELL),
                        in1=bc_sb[:].unsqueeze(1).to_broadcast((P, NT, NCELL)),
                        op=OP.add)
                nc.vector.memset(l8[:], -1e30)
                nc.vector.tensor_copy(
                    out=l8[:].rearrange("p (t e) -> p t e", e=8)[:, :, 0:NCELL],
                    in_=lg[:].rearrange("p (t n) -> p t n", n=NCELL))
                for t_ in range(NT):
                    nc.vector.max(mx8[:, t_ * 8:(t_ + 1) * 8],
                                  l8[:, t_ * 8:(t_ + 1) * 8])
                    nc.vector.max_index(ix8[:, t_ * 8:(t_ + 1) * 8],
                                        mx8[:, t_ * 8:(t_ + 1) * 8],
                                        l8[:, t_ * 8:(t_ + 1) * 8])
                mx_v = mx8[:].rearrange("p (t e) -> p t e", e=8)
                ix_v = ix8[:].rearrange("p (t e) -> p t e", e=8)
                nc.vector.tensor_tensor(out=diff[:].unsqueeze(2),
                                        in0=mx_v[:, :, 0:1], in1=mx_v[:, :, 1:2],
                                        op=OP.subtract)
                nc.scalar.activation(p1[:], diff[:], AF.Sigmoid)
                nc.vector.tensor_scalar(p2[:], p1[:], -1.0, 1.0, OP.mult, OP.add)
                nc.vector.tensor_copy(i1f[:].unsqueeze(2), ix_v[:, :, 0:1])
                nc.vector.tensor_copy(i2f[:].unsqueeze(2), ix_v[:, :, 1:2])
                nc.gpsimd.iota(iota_i[:], pattern=[[0, NT], [1, NCELL]],
                               base=0, channel_multiplier=0)
                nc.vector.tensor_copy(iota_f[:], iota_i[:])
                iota_v = iota_f[:].rearrange("p (t n) -> p t n", n=NCELL)
                gate_v = gate[:].rearrange("p (t n) -> p t n", n=NCELL)
                g2_v = g2[:].rearrange("p (t n) -> p t n", n=NCELL)
                nc.vector.tensor_tensor(
                    out=gate_v,
                    in0=i1f[:].unsqueeze(2).to_broadcast((P, NT, NCELL)),
                    in1=iota_v, op=OP.is_equal)
                nc.vector.tensor_tensor(
                    out=gate_v, in0=gate_v,
                    in1=p1[:].unsqueeze(2).to_broadcast((P, NT, NCELL)), op=OP.mult)
                nc.vector.tensor_tensor(
                    out=g2_v,
                    in0=i2f[:].unsqueeze(2).to_broadcast((P, NT, NCELL)),
                    in1=iota_v, op=OP.is_equal)
                nc.vector.tensor_tensor(
                    out=g2_v, in0=g2_v,
                    in1=p2[:].unsqueeze(2).to_broadcast((P, NT, NCELL)), op=OP.mult)
                nc.vector.tensor_tensor(out=gate_v, in0=gate_v, in1=g2_v, op=OP.add)

            # ---- phase B: dense gate matmul (bf16) + LSTM math + combine ----
            # act layout per batch tile: [i(1024) | f(1024) | o(1024) | tanh(j)(1024)]
            # (tanh(j) slot is later overwritten with tanh(new_c));
            # ncnh layout: [new_c(1024) | new_h(1024)]
            # Engine streams are in-order, so thc/new_h run 1 tile behind the
            # matmuls and the routed combine 2 tiles behind; the gate chain is
            # emitted after bt1 so nothing ever waits on it.
            acts = [None] * NT
            ncnhs = [None] * NT

            def emit_thc_newh(j_):
                thc = acts[j_][:, 3072:4096]
                nc.scalar.activation(thc, ncnhs[j_][:, 0:1024], AF.Tanh)
                nc.vector.tensor_tensor(out=ncnhs[j_][:, 1024:2048],
                                        in0=acts[j_][:, 2048:3072], in1=thc,
                                        op=OP.mult)

            def emit_combine(j_):
                acc = work.tile([P, 2 * OUT], F32, tag="acc", name=f"acc{j_}",
                                bufs=3)
                acc_v = acc[:].rearrange("p (u o) -> p u o", o=OUT)
                src = ncnhs[j_][:].rearrange("p (u n o) -> p n u o", o=OUT, u=2)
                nc.vector.tensor_scalar(
                    acc_v, src[:, 0], gate[:, j_ * NCELL:j_ * NCELL + 1],
                    None, OP.mult)
                for n_ in range(1, NCELL):
                    nc.vector.scalar_tensor_tensor(
                        out=acc_v, in0=src[:, n_],
                        scalar=gate[:, j_ * NCELL + n_:j_ * NCELL + n_ + 1],
                        in1=acc_v, op0=OP.mult, op1=OP.add)
                nc.sync.dma_start(out=nc_o[j_ * P:(j_ + 1) * P, :],
                                  in_=acc[:, 0:OUT])
                nc.sync.dma_start(out=nh_o[j_ * P:(j_ + 1) * P, :],
                                  in_=acc[:, OUT:2 * OUT])

            with tc.tile_pool(name="psB", bufs=2, space="PSUM") as psB:
                for t_ in range(NT):
                    act = work.tile([P, GC], EW, tag="act", name=f"act{t_}",
                                    bufs=2)
                    acts[t_] = act
                    for half in range(2):
                        ps = psB.tile([P, 2048], F32, tag="mm", name=f"mm{t_}_{half}")
                        for k in range(KT):
                            lhs = fTb_sb[:, k * BL + t_ * P:k * BL + (t_ + 1) * P]
                            for c4 in range(4):
                                col = half * 2048 + c4 * 512
                                nc.tensor.matmul(
                                    ps[:, c4 * 512:(c4 + 1) * 512],
                                    lhsT=lhs,
                                    rhs=wg_sb[:, k * GC + col:k * GC + col + 512],
                                    start=(k == 0), stop=(k == KT - 1))
                        if has_bg:
                            nc.vector.tensor_tensor(
                                out=ps[:], in0=ps[:],
                                in1=bg_sb[:, half * 2048:(half + 1) * 2048],
                                op=OP.add)
                        if half == 0:
                            nc.scalar.activation(act[:, 0:2048], ps[:], AF.Sigmoid)
                        else:
                            nc.scalar.activation(act[:, 2048:3072], ps[:, 0:1024],
                                                 AF.Sigmoid)
                            nc.scalar.activation(act[:, 3072:4096], ps[:, 1024:2048],
                                                 AF.Tanh)
                    if t_ >= 1:
                        emit_thc_newh(t_ - 1)

                    tij = work.tile([P, NCELL * OUT], EW, tag="tij",
                                    name=f"tij{t_}")
                    nc.vector.tensor_tensor(out=tij[:], in0=act[:, 0:1024],
                                            in1=act[:, 3072:4096], op=OP.mult)
                    ncnh = work.tile([P, 2 * NCELL * OUT], EW, tag="ncnh",
                                     name=f"ncnh{t_}", bufs=7)
                    ncnhs[t_] = ncnh
                    c_bt = c_sb[:, t_ * OUT:(t_ + 1) * OUT]
                    nc.vector.tensor_tensor(
                        out=ncnh[:, 0:1024].rearrange("p (n o) -> p n o", o=OUT),
                        in0=act[:, 1024:2048].rearrange("p (n o) -> p n o", o=OUT),
                        in1=c_bt.unsqueeze(1).to_broadcast((P, NCELL, OUT)),
                        op=OP.mult)
                    nc.vector.tensor_tensor(out=ncnh[:, 0:1024],
                                            in0=ncnh[:, 0:1024], in1=tij[:],
                                            op=OP.add)
                    if t_ == 4:
                        lg_ps = psB.tile([P, 2048], F32, tag="mm", name="mm_lg")
                        emit_logits_and_gates(lg_ps)
                    if t_ >= 6:
                        emit_combine(t_ - 6)

                emit_thc_newh(NT - 1)
                for j_ in range(NT - 6, NT):
                    emit_combine(j_)
    nc.compile()
    return nc


_programs = {}


def _get_program(has_bg, has_bc):
    key = (has_bg, has_bc)
    if key not in _programs:
        _programs[key] = _build_program(has_bg, has_bc)
    return _programs[key]


def kernel(x, c, h, W_gates, b_gates, W_ctrl, b_ctrl):
    global LAST_RESULTS
    x = np.ascontiguousarray(np.asarray(x, dtype=np.float32))
    c = np.ascontiguousarray(np.asarray(c, dtype=np.float32))
    h = np.ascontiguousarray(np.asarray(h, dtype=np.float32))
    W_gates = np.asarray(W_gates, dtype=np.float32)
    b_gates = np.asarray(b_gates, dtype=np.float32)
    W_ctrl = np.ascontiguousarray(np.asarray(W_ctrl, dtype=np.float32))
    b_ctrl = np.asarray(b_ctrl, dtype=np.float32)

    featsT = np.ascontiguousarray(np.concatenate([x, h], axis=1).T)  # [D, B]
    # permute W_gates columns [d, n, g, o] -> gate-major [d, (i,f,o,j), n, o]
    wg_p = np.ascontiguousarray(
        W_gates.reshape(D, NCELL, 4, OUT)[:, :, [0, 2, 3, 1], :]
        .transpose(0, 2, 1, 3).reshape(D, GC))
    bg_p = np.ascontiguousarray(
        b_gates.reshape(NCELL, 4, OUT)[:, [0, 2, 3, 1], :]
        .transpose(1, 0, 2).reshape(1, GC))

    import ml_dtypes
    featsTb = featsT.astype(ml_dtypes.bfloat16)
    wg_b = wg_p.astype(ml_dtypes.bfloat16)
    c_dev = c.astype(ml_dtypes.bfloat16) if EW_BF16 else c

    has_bg = bool(np.any(b_gates))
    has_bc = bool(np.any(b_ctrl))
    prog = _get_program(has_bg, has_bc)

    in_maps = []
    for i in range(N_CORES):
        m = {
            "featsT": np.ascontiguousarray(featsT[:, i * BL:(i + 1) * BL]),
            "featsTb": np.ascontiguousarray(featsTb[:, i * BL:(i + 1) * BL]),
            "c_in": np.ascontiguousarray(c_dev[i * BL:(i + 1) * BL]),
            "wgb": wg_b,
            "wc": W_ctrl,
        }
        if has_bg:
            m["bg"] = bg_p
        if has_bc:
            m["bc"] = np.ascontiguousarray(b_ctrl.reshape(1, NCELL))
        in_maps.append(m)

    res = run_bass_kernel_spmd(prog, in_maps, core_ids=list(range(N_CORES)),
                               trace=TRACE)
    LAST_RESULTS = res
    nh = np.concatenate([res.results[i]["nh_out"] for i in range(N_CORES)], axis=0)
    ncv = np.concatenate([res.results[i]["nc_out"] for i in range(N_CORES)], axis=0)
    return nh.astype(np.float32), ncv.astype(np.float32)


# revision 14
# speedup vs baseline: 1.3468x; 1.0007x over previous
"""Trainium2 Bass kernel for a top-2-of-4 routed LSTM cell bank (MoE routing).

Reference computation (per batch row b):
    feats    = concat(x[b], h[b])                      # [512]
    logits   = feats @ W_ctrl + b_ctrl                 # [4]
    gate     = top2_softmax(logits)                    # [4], 2 nonzero
    combined = feats @ W_gates + b_gates               # [4 cells, 4 gates, 256]
    i, j, f, o = gates;  new_c_n = sig(f)*c + sig(i)*tanh(j);  new_h_n = sig(o)*tanh(new_c_n)
    nh[b] = sum_n gate[n]*new_h_n ; nc[b] = sum_n gate[n]*new_c_n

Strategy: data-parallel over 8 NeuronCores (2048 batch rows each), weights
replicated.  Per core everything is dense and batch-tiled (16 tiles of 128
rows).  The routing logits are computed with true-fp32 matmuls (the smallest
top2/top3 logit gap in the dataset is ~2e-5, so reduced precision would flip
routing decisions); the big [2048,512]@[512,4096] gate matmul runs in bf16
(full PE stream rate; fp32 matmul is 4 cycles/row).  W_gates columns are
permuted host-side to gate-major [i|f|o|j] order so each activation function
covers one contiguous span per batch tile.  The heavy elementwise chain is
split across DVE and GPSIMD; new_c/new_h live in one tile so the routed
combine handles both outputs per instruction.
"""

import sys

for _p in ("/opt/trn_rl_repo", "/root/.axon_site/_ro/trn_rl_repo"):
    if _p not in sys.path:
        sys.path.append(_p)

import numpy as np

import concourse.bacc as bacc
from concourse import bass, mybir
from concourse.bass_utils import run_bass_kernel_spmd
from concourse.tile import TileContext

P = 128
N_CORES = 8
B = 16384
IN = 256
OUT = 256
NCELL = 4
D = IN + OUT          # 512
KT = D // P           # 4 contraction tiles
BL = B // N_CORES     # 2048 rows per core
NT = BL // P          # 16 batch tiles per core
GC = 4 * OUT * NCELL  # 4096 gate columns

F32 = mybir.dt.float32
BF16 = mybir.dt.bfloat16
U32 = mybir.dt.uint32
I32 = mybir.dt.int32
AF = mybir.ActivationFunctionType
OP = mybir.AluOpType

# elementwise intermediates in bf16: 2x DVE throughput, half the SBUF
EW_BF16 = True

# test.py can flip these to capture a profiled run
TRACE = False
LAST_RESULTS = None


def _build_program(has_bg: bool, has_bc: bool):
    nc = bacc.Bacc("TRN2", target_bir_lowering=False, debug=False,
                   num_devices=N_CORES)

    featsT = nc.dram_tensor("featsT", (D, BL), F32, kind="ExternalInput").ap()
    featsTb = nc.dram_tensor("featsTb", (D, BL), BF16, kind="ExternalInput").ap()
    wgb = nc.dram_tensor("wgb", (D, GC), BF16, kind="ExternalInput").ap()
    c_in = nc.dram_tensor("c_in", (BL, OUT), BF16 if EW_BF16 else F32,
                          kind="ExternalInput").ap()
    wc = nc.dram_tensor("wc", (D, NCELL), F32, kind="ExternalInput").ap()
    bg = bc = None
    if has_bg:
        bg = nc.dram_tensor("bg", (1, GC), F32, kind="ExternalInput").ap()
    if has_bc:
        bc = nc.dram_tensor("bc", (1, NCELL), F32, kind="ExternalInput").ap()
    nh_o = nc.dram_tensor("nh_out", (BL, OUT), F32, kind="ExternalOutput").ap()
    nc_o = nc.dram_tensor("nc_out", (BL, OUT), F32, kind="ExternalOutput").ap()

    with TileContext(nc) as tc:
        with tc.tile_pool(name="const", bufs=1) as konst, \
             tc.tile_pool(name="work", bufs=2) as work:

            # ---- input loads; one HWDGE FIFO gives strict priority order:
            # bf16 featsT -> W halves -> c -> fp32 featsT (logits late is fine)
            EW = BF16 if EW_BF16 else F32
            fTb_sb = konst.tile([P, KT * BL], BF16, tag="fTb")
            nc.sync.dma_start(
                out=fTb_sb[:].rearrange("p (k b) -> p k b", b=BL),
                in_=featsTb.rearrange("(k p) b -> p k b", p=P))
            wg_sb = konst.tile([P, KT * GC], BF16, tag="wg")
            wg_v = wg_sb[:].rearrange("p (k n) -> p k n", n=GC)
            wg_src = wgb.rearrange("(k p) n -> p k n", p=P)
            for half in range(2):
                for kp in range(2):
                    nc.sync.dma_start(
                        out=wg_v[:, 2 * kp:2 * kp + 2, half * 2048:(half + 1) * 2048],
                        in_=wg_src[:, 2 * kp:2 * kp + 2, half * 2048:(half + 1) * 2048])
            c_sb = konst.tile([P, NT * OUT], EW, tag="c")
            nc.sync.dma_start(
                out=c_sb[:].rearrange("p (t o) -> p t o", o=OUT),
                in_=c_in.rearrange("(t p) o -> p t o", p=P))
            fT_sb = konst.tile([P, KT * BL], F32, tag="fT")
            nc.sync.dma_start(
                out=fT_sb[:].rearrange("p (k b) -> p k b", b=BL),
                in_=featsT.rearrange("(k p) b -> p k b", p=P))
            wc_sb = konst.tile([P, KT * NCELL], F32, tag="wc")
            nc.sync.dma_start(
                out=wc_sb[:].rearrange("p (k n) -> p k n", n=NCELL),
                in_=wc.rearrange("(k p) n -> p k n", p=P))
            bg_sb = bc_sb = None
            if has_bg:
                bg_sb = konst.tile([P, GC], F32, tag="bg")
                nc.sync.dma_start(out=bg_sb[:], in_=bg.partition_broadcast(P)[:, 0, :])
            if has_bc:
                bc_sb = konst.tile([P, NCELL], F32, tag="bc")
                nc.sync.dma_start(out=bc_sb[:], in_=bc.partition_broadcast(P)[:, 0, :])

            # gate-phase tiles (filled mid-loop, after bt1's matmuls)
            lg = konst.tile([P, NT * NCELL], F32, tag="lg")
            l8 = konst.tile([P, NT * 8], F32, tag="l8")
            mx8 = konst.tile([P, NT * 8], F32, tag="mx8")
            ix8 = konst.tile([P, NT * 8], U32, tag="ix8")
            diff = konst.tile([P, NT], F32, tag="diff")
            p1 = konst.tile([P, NT], F32, tag="p1")
            p2 = konst.tile([P, NT], F32, tag="p2")
            i1f = konst.tile([P, NT], F32, tag="i1f")
            i2f = konst.tile([P, NT], F32, tag="i2f")
            iota_i = konst.tile([P, NT * NCELL], I32, tag="iota_i")
            iota_f = konst.tile([P, NT * NCELL], F32, tag="iota_f")
            gate = konst.tile([P, NT * NCELL], F32, tag="gate")
            g2 = konst.tile([P, NT * NCELL], F32, tag="g2")

            def emit_logits_and_gates(lg_ps):
                # routing logits: true fp32 matmuls (reduced precision would
                # flip top-2 decisions; min top2/top3 gap here is ~2e-5)
                for t_ in range(NT):
                    for k in range(KT):
                        nc.tensor.matmul(
                            lg_ps[:, t_ * NCELL:(t_ + 1) * NCELL],
                            lhsT=fT_sb[:, k * BL + t_ * P:k * BL + (t_ + 1) * P],
                            rhs=wc_sb[:, k * NCELL:(k + 1) * NCELL],
                            start=(k == 0), stop=(k == KT - 1))
                nc.vector.tensor_copy(lg[:], lg_ps[:, 0:NT * NCELL])
                if has_bc:
                    nc.vector.tensor_tensor(
                        out=lg[:].rearrange("p (t n) -> p t n", n=NCELL),
                        in0=lg[:].rearrange("p (t n) -> p t n", n=NCELL),
                        in1=bc_sb[:].unsqueeze(1).to_broadcast((P, NT, NCELL)),
                        op=OP.add)
                nc.vector.memset(l8[:], -1e30)
                nc.vector.tensor_copy(
                    out=l8[:].rearrange("p (t e) -> p t e", e=8)[:, :, 0:NCELL],
                    in_=lg[:].rearrange("p (t n) -> p t n", n=NCELL))
                for t_ in range(NT):
                    nc.vector.max(mx8[:, t_ * 8:(t_ + 1) * 8],
                                  l8[:, t_ * 8:(t_ + 1) * 8])
                    nc.vector.max_index(ix8[:, t_ * 8:(t_ + 1) * 8],
                                        mx8[:, t_ * 8:(t_ + 1) * 8],
                                        l8[:, t_ * 8:(t_ + 1) * 8])
                mx_v = mx8[:].rearrange("p (t e) -> p t e", e=8)
                ix_v = ix8[:].rearrange("p (t e) -> p t e", e=8)
                nc.vector.tensor_tensor(out=diff[:].unsqueeze(2),
                                        in0=mx_v[:, :, 0:1], in1=mx_v[:, :, 1:2],
                                        op=OP.subtract)
                nc.scalar.activation(p1[:], diff[:], AF.Sigmoid)
                nc.vector.tensor_scalar(p2[:], p1[:], -1.0, 1.0, OP.mult, OP.add)
                nc.vector.tensor_copy(i1f[:].unsqueeze(2), ix_v[:, :, 0:1])
                nc.vector.tensor_copy(i2f[:].unsqueeze(2), ix_v[:, :, 1:2])
                nc.gpsimd.iota(iota_i[:], pattern=[[0, NT], [1, NCELL]],
                               base=0, channel_multiplier=0)
                nc.vector.tensor_copy(iota_f[:], iota_i[:])
                iota_v = iota_f[:].rearrange("p (t n) -> p t n", n=NCELL)
                gate_v = gate[:].rearrange("p (t n) -> p t n", n=NCELL)
                g2_v = g2[:].rearrange("p (t n) -> p t n", n=NCELL)
                nc.vector.tensor_tensor(
                    out=gate_v,
                    in0=i1f[:].unsqueeze(2).to_broadcast((P, NT, NCELL)),
                    in1=iota_v, op=OP.is_equal)
                nc.vector.tensor_tensor(
                    out=gate_v, in0=gate_v,
                    in1=p1[:].unsqueeze(2).to_broadcast((P, NT, NCELL)), op=OP.mult)
                nc.vector.tensor_tensor(
                    out=g2_v,
                    in0=i2f[:].unsqueeze(2).to_broadcast((P, NT, NCELL)),
                    in1=iota_v, op=OP.is_equal)
                nc.vector.tensor_tensor(
                    out=g2_v, in0=g2_v,
                    in1=p2[:].unsqueeze(2).to_broadcast((P, NT, NCELL)), op=OP.mult)
                nc.vector.tensor_tensor(out=gate_v, in0=gate_v, in1=g2_v, op=OP.add)

            # ---- phase B: dense gate matmul (bf16) + LSTM math + combine ----
            # act layout per batch tile: [i(1024) | f(1024) | o(1024) | tanh(j)(1024)]
            # (tanh(j) slot is later overwritten with tanh(new_c));
            # ncnh layout: [new_c(1024) | new_h(1024)]
            # Engine streams are in-order, so thc/new_h run 1 tile behind the
            # matmuls and the routed combine 2 tiles behind; the gate chain is
            # emitted after bt1 so nothing ever waits on it.
            acts = [None] * NT
            ncnhs = [None] * NT

            def emit_thc_newh(j_):
                thc = acts[j_][:, 3072:4096]
                nc.scalar.activation(thc, ncnhs[j_][:, 0:1024], AF.Tanh)
                nc.vector.tensor_tensor(out=ncnhs[j_][:, 1024:2048],
                                        in0=acts[j_][:, 2048:3072], in1=thc,
                                        op=OP.mult)

            def emit_combine(j_):
                acc = work.tile([P, 2 * OUT], F32, tag="acc", name=f"acc{j_}",
                                bufs=3)
                acc_v = acc[:].rearrange("p (u o) -> p u o", o=OUT)
                src = ncnhs[j_][:].rearrange("p (u n o) -> p n u o", o=OUT, u=2)
                nc.vector.tensor_scalar(
                    acc_v, src[:, 0], gate[:, j_ * NCELL:j_ * NCELL + 1],
                    None, OP.mult)
                for n_ in range(1, NCELL):
                    nc.vector.scalar_tensor_tensor(
                        out=acc_v, in0=src[:, n_],
                        scalar=gate[:, j_ * NCELL + n_:j_ * NCELL + n_ + 1],
                        in1=acc_v, op0=OP.mult, op1=OP.add)
                nc.sync.dma_start(out=nc_o[j_ * P:(j_ + 1) * P, :],
                                  in_=acc[:, 0:OUT])
                nc.sync.dma_start(out=nh_o[j_ * P:(j_ + 1) * P, :],
                                  in_=acc[:, OUT:2 * OUT])

            with tc.tile_pool(name="psB", bufs=2, space="PSUM") as psB:
                for t_ in range(NT):
                    act = work.tile([P, GC], EW, tag="act", name=f"act{t_}",
                                    bufs=2)
                    acts[t_] = act
                    for half in range(2):
                        ps = psB.tile([P, 2048], F32, tag="mm", name=f"mm{t_}_{half}")
                        for k in range(KT):
                            lhs = fTb_sb[:, k * BL + t_ * P:k * BL + (t_ + 1) * P]
                            for c4 in range(4):
                                col = half * 2048 + c4 * 512
                                nc.tensor.matmul(
                                    ps[:, c4 * 512:(c4 + 1) * 512],
                                    lhsT=lhs,
                                    rhs=wg_sb[:, k * GC + col:k * GC + col + 512],
                                    start=(k == 0), stop=(k == KT - 1))
                        if has_bg:
                            nc.vector.tensor_tensor(
                                out=ps[:], in0=ps[:],
                                in1=bg_sb[:, half * 2048:(half + 1) * 2048],
                                op=OP.add)
                        if half == 0:
                            nc.scalar.activation(act[:, 0:2048], ps[:], AF.Sigmoid)
                        else:
                            nc.scalar.activation(act[:, 2048:3072], ps[:, 0:1024],
                                                 AF.Sigmoid)
                            nc.scalar.activation(act[:, 3072:4096], ps[:, 1024:2048],
                                                 AF.Tanh)
                    if t_ >= 1:
                        emit_thc_newh(t_ - 1)

                    tij = work.tile([P, NCELL * OUT], EW, tag="tij",
                                    name=f"tij{t_}")
                    nc.vector.tensor_tensor(out=tij[:], in0=act[:, 0:1024],
                                            in1=act[:, 3072:4096], op=OP.mult)
                    ncnh = work.tile([P, 2 * NCELL * OUT], EW, tag="ncnh",
                                     name=f"ncnh{t_}", bufs=6)
                    ncnhs[t_] = ncnh
                    c_bt = c_sb[:, t_ * OUT:(t_ + 1) * OUT]
                    nc.vector.tensor_tensor(
                        out=ncnh[:, 0:1024].rearrange("p (n o) -> p n o", o=OUT),
                        in0=act[:, 1024:2048].rearrange("p (n o) -> p n o", o=OUT),
                        in1=c_bt.unsqueeze(1).to_broadcast((P, NCELL, OUT)),
                        op=OP.mult)
                    nc.vector.tensor_tensor(out=ncnh[:, 0:1024],
                                            in0=ncnh[:, 0:1024], in1=tij[:],
                                            op=OP.add)
                    if t_ == 2:
                        lg_ps = psB.tile([P, 2048], F32, tag="mm", name="mm_lg")
                        emit_logits_and_gates(lg_ps)
                    if t_ >= 5:
                        emit_combine(t_ - 5)

                emit_thc_newh(NT - 1)
                for j_ in range(NT - 5, NT):
                    emit_combine(j_)
    nc.compile()
    return nc


_programs = {}


def _get_program(has_bg, has_bc):
    key = (has_bg, has_bc)
    if key not in _programs:
        _programs[key] = _build_program(has_bg, has_bc)
    return _programs[key]


def kernel(x, c, h, W_gates, b_gates, W_ctrl, b_ctrl):
    global LAST_RESULTS
    x = np.ascontiguousarray(np.asarray(x, dtype=np.float32))
    c = np.ascontiguousarray(np.asarray(c, dtype=np.float32))
    h = np.ascontiguousarray(np.asarray(h, dtype=np.float32))
    W_gates = np.asarray(W_gates, dtype=np.float32)
    b_gates = np.asarray(b_gates, dtype=np.float32)
    W_ctrl = np.ascontiguousarray(np.asarray(W_ctrl, dtype=np.float32))
    b_ctrl = np.asarray(b_ctrl, dtype=np.float32)

    featsT = np.ascontiguousarray(np.concatenate([x, h], axis=1).T)  # [D, B]
    # permute W_gates columns [d, n, g, o] -> gate-major [d, (i,f,o,j), n, o]
    wg_p = np.ascontiguousarray(
        W_gates.reshape(D, NCELL, 4, OUT)[:, :, [0, 2, 3, 1], :]
        .transpose(0, 2, 1, 3).reshape(D, GC))
    bg_p = np.ascontiguousarray(
        b_gates.reshape(NCELL, 4, OUT)[:, [0, 2, 3, 1], :]
        .transpose(1, 0, 2).reshape(1, GC))

    import ml_dtypes
    featsTb = featsT.astype(ml_dtypes.bfloat16)
    wg_b = wg_p.astype(ml_dtypes.bfloat16)
    c_dev = c.astype(ml_dtypes.bfloat16) if EW_BF16 else c

    has_bg = bool(np.any(b_gates))
    has_bc = bool(np.any(b_ctrl))
    prog = _get_program(has_bg, has_bc)

    in_maps = []
    for i in range(N_CORES):
        m = {
            "featsT": np.ascontiguousarray(featsT[:, i * BL:(i + 1) * BL]),
            "featsTb": np.ascontiguousarray(featsTb[:, i * BL:(i + 1) * BL]),
            "c_in": np.ascontiguousarray(c_dev[i * BL:(i + 1) * BL]),
            "wgb": wg_b,
            "wc": W_ctrl,
        }
        if has_bg:
            m["bg"] = bg_p
        if has_bc:
            m["bc"] = np.ascontiguousarray(b_ctrl.reshape(1, NCELL))
        in_maps.append(m)

    res = run_bass_kernel_spmd(prog, in_maps, core_ids=list(range(N_CORES)),
                               trace=TRACE)
    LAST_RESULTS = res
    nh = np.concatenate([res.results[i]["nh_out"] for i in range(N_CORES)], axis=0)
    ncv = np.concatenate([res.results[i]["nc_out"] for i in range(N_CORES)], axis=0)
    return nh.astype(np.float32), ncv.astype(np.float32)
